# revision 13
# baseline (speedup 1.0000x reference)
"""Trainium2 Bass kernel for nn_Decoder (GRU decoder + Luong attention + greedy decode).

Strategy (8 NeuronCores, SPMD, same NEFF, per-core data):
  - GRU + tanh-output replicated for the full batch B=64 (matmul streams are
    B-independent); attention batch-sharded 8 ways (dual enc layouts per core);
    vocab projection sharded 8 ways (fc rows 4000/core, float32r matmuls).
  - Per step two tiny AllGathers: (1) o_t shards [8,512] -> full o_t, (2) local
    argmax partials (+ deferred softmax stats) -> greedy token feedback.
  - log_softmax normalization is deferred one step and runs off the critical
    path; ln() is computed on DVE via exponent/mantissa split + Chebyshev
    polynomial so the ACT engine only ever uses the exp/tanh table set.
"""

import os
from contextlib import ExitStack

import numpy as np

import concourse.bass as bass
import concourse.bacc as bacc
import concourse.mybir as mybir
from concourse.tile import TileContext
from concourse.bass import IndirectOffsetOnAxis
from concourse.bass_utils import run_bass_kernel_spmd

F32 = mybir.dt.float32
F32R = mybir.dt.float32r
U32 = mybir.dt.uint32

V = 32000
E = 256
H = 512
EH = 512
B = 64
T_ENC = 128
SOS = 2
NCORE = 8
BS = B // NCORE        # 8 batches per core (attention shard)
VS = V // NCORE        # 4000 vocab rows per core
ZC = 500               # z free-dim chunk (VS = 8 * ZC)
NZC = VS // ZC
LN2 = 0.6931471805599453

_LAST_RESULTS = None   # BassKernelResults of the most recent run (for test.py)


def _log2_cheb_coeffs(deg=12):
    # Chebyshev fit of log2 on [1, 2] -> power-series coeffs (ascending in f).
    x = np.cos(np.pi * (np.arange(200) + 0.5) / 200)
    xs = 1.5 + 0.5 * x
    c = np.polynomial.chebyshev.chebfit(x, np.log2(xs), deg)
    p = np.polynomial.chebyshev.cheb2poly(c)
    q = np.polynomial.polynomial.Polynomial(p)(
        np.polynomial.polynomial.Polynomial([-3.0, 2.0]))
    return q.coef


def _kmajor(w):
    """[K, N] (K multiple of 128) -> [128, (K//128)*N]; chunk k at cols [k*N:(k+1)*N]."""
    K, N = w.shape
    assert K % 128 == 0
    return np.ascontiguousarray(
        w.reshape(K // 128, 128, N).transpose(1, 0, 2).reshape(128, (K // 128) * N)
    ).astype(np.float32)


def _emit(nc, T_DEC):
    AF = mybir.ActivationFunctionType
    ALU = mybir.AluOpType
    AX = mybir.AxisListType
    RG = [list(range(NCORE))]

    ei = lambda name, shape, d=F32: nc.dram_tensor(name, shape, d, kind="ExternalInput")
    emb_d = ei("emb", [V, E])
    encselT_d = ei("encselT", [128, 4 * BS * T_ENC])
    encselN_d = ei("encselN", [128, BS * EH])
    fcT_d = ei("fcT", [128, 4 * VS])
    whhT_d = ei("whhT", [128, 4 * 1536])
    wihT_d = ei("wihT", [128, 2 * 1536])
    waattn_d = ei("waattn", [128, 4 * EH])
    waT_d = ei("waT", [128, 8 * H])
    bias_rz_d = ei("bias_rz", [B, 1024])
    bias_hn_d = ei("bias_hn", [B, H])
    bias_in_d = ei("bias_in", [B, H])
    ident_d = ei("ident", [128, 128])
    sel_d = ei("sel", [B, BS])
    scmask_d = ei("scmask", [BS, BS * T_ENC])
    voff_d = ei("voff", [B, 1])
    h0_d = ei("h0", [B, H])
    h0T_d = ei("h0T", [128, 4 * B])
    x0T_d = ei("x0T", [128, 2 * B])

    logp_d = nc.dram_tensor("logp", [B, T_DEC, VS], F32, kind="ExternalOutput")
    hfin_d = nc.dram_tensor("hfin", [B, H], F32, kind="ExternalOutput")
    zraw_d = nc.dram_tensor("zraw", [T_DEC, B, VS], F32, kind="Internal")

    r32 = lambda ap: ap  # fp32r is TF32 (10-bit mantissa) - too imprecise here
    cheb = _log2_cheb_coeffs(12)[::-1]  # descending for Horner

    with TileContext(nc) as tc, ExitStack() as ctx:
        cst = ctx.enter_context(tc.tile_pool(name="cst", bufs=1))
        wrk = ctx.enter_context(tc.tile_pool(name="wrk", bufs=1))
        psA = ctx.enter_context(tc.tile_pool(name="psA", bufs=2, space="PSUM"))
        psB = ctx.enter_context(tc.tile_pool(name="psB", bufs=3, space="PSUM"))
        drm = ctx.enter_context(tc.tile_pool(name="drm", bufs=2, space="DRAM"))

        def load_const(dram, shape):
            t = cst.tile(shape, F32, name=dram.name + "_sb")
            nc.sync.dma_start(t[:], dram.ap())
            return t

        encselT = load_const(encselT_d, [128, 4 * BS * T_ENC])
        encselN = load_const(encselN_d, [128, BS * EH])
        fcT = load_const(fcT_d, [128, 4 * VS])
        whhT = load_const(whhT_d, [128, 4 * 1536])
        wihT = load_const(wihT_d, [128, 2 * 1536])
        waattn = load_const(waattn_d, [128, 4 * EH])
        waT = load_const(waT_d, [128, 8 * H])
        bias_rz = load_const(bias_rz_d, [B, 1024])
        bias_hn = load_const(bias_hn_d, [B, H])
        bias_in = load_const(bias_in_d, [B, H])
        ident = load_const(ident_d, [128, 128])
        sel = load_const(sel_d, [B, BS])
        scmask = load_const(scmask_d, [BS, BS * T_ENC])
        voff = load_const(voff_d, [B, 1])
        h0 = wrk.tile([B, H], F32, name="h_new", tag="h_new", bufs=2)
        nc.sync.dma_start(h0[:], h0_d.ap())
        h0T = wrk.tile([128, 4 * B], F32, name="hT_n", tag="hT_n", bufs=2)
        nc.sync.dma_start(h0T[:], h0T_d.ap())
        x0T = wrk.tile([128, 2 * B], F32, name="xT_n", tag="xT_n", bufs=2)
        nc.sync.dma_start(x0T[:], x0T_d.ap())

        # small-psum slot: shared by transposes / sel / ctx / o / z chunks
        def small_ps(p, f):
            t = psB.tile([128, 512], F32, name="sps", tag="sps")
            return t[:p, :f]

        def w2(shape, name, dtype=F32, bufs=1):
            return wrk.tile(shape, dtype, name=name, tag=name, bufs=bufs)

        def pe_tr(src_ap, p, f, dst_ap):
            """dst[f-cols...] = src[p,f].T via PE; dst is an SBUF slice [f, p]."""
            ps = small_ps(f, p)
            nc.tensor.transpose(ps, src_ap, ident[:p, :p])
            nc.vector.tensor_copy(dst_ap, ps)

        def ln_into(out_ap, s_ap, uniq):
            """out = ln(s) elementwise on [B,1] via bits + Chebyshev log2."""
            ubits = w2([B, 1], "ubits" + uniq, U32)
            nc.vector.tensor_copy(ubits[:].bitcast(F32), s_ap)
            eu = w2([B, 1], "eu" + uniq, U32)
            nc.vector.tensor_scalar(eu[:], ubits[:], 23, None,
                                    ALU.logical_shift_right)
            ef = w2([B, 1], "ef" + uniq)
            nc.vector.tensor_copy(ef[:], eu[:])
            mu = w2([B, 1], "mu" + uniq, U32)
            nc.vector.tensor_scalar(mu[:], ubits[:], 0x007FFFFF, 0x3F800000,
                                    ALU.bitwise_and, ALU.bitwise_or)
            mf = mu[:].bitcast(F32)
            nc.vector.tensor_scalar(out_ap, mf, float(cheb[0]), float(cheb[1]),
                                    ALU.mult, ALU.add)
            for c in cheb[2:]:
                nc.vector.tensor_scalar(out_ap, out_ap, mf, float(c),
                                        ALU.mult, ALU.add)
            nc.vector.tensor_add(out_ap, out_ap, ef[:])
            nc.vector.tensor_scalar(out_ap, out_ap, -127.0, LN2, ALU.add, ALU.mult)

        def normalize_block(t_src, vals_prev_ap, gmax_prev_ap, sumexps_ap, uniq):
            """logp[:, t_src, :] = zraw[t_src] - (gmax + ln(sum_c S_c e^{lm_c-gmax}))."""
            adj = w2([B, NCORE], "adj" + uniq)
            ngp = w2([B, 1], "ngp" + uniq)
            nc.vector.tensor_scalar(ngp[:], gmax_prev_ap, -1.0, None, ALU.mult)
            nc.scalar.activation(adj[:], vals_prev_ap, AF.Exp, bias=ngp[:])
            nc.vector.tensor_mul(adj[:], adj[:], sumexps_ap)
            stot = w2([B, 1], "stot" + uniq)
            nc.vector.tensor_reduce(stot[:], adj[:], AX.X, ALU.add)
            logz = w2([B, 1], "logz" + uniq)
            ln_into(logz[:], stot[:], uniq)
            nc.vector.tensor_add(logz[:], logz[:], gmax_prev_ap)
            for jn in range(8):
                zn = w2([B, 500], "zn", bufs=1)
                nc.sync.dma_start(
                    zn[:], zraw_d.ap()[t_src, :, jn * 500:(jn + 1) * 500])
                nc.vector.tensor_scalar(zn[:], zn[:], logz[:], None, ALU.subtract)
                nc.sync.dma_start(
                    logp_d.ap()[:, t_src, jn * 500:(jn + 1) * 500], zn[:])

        # ---- state carried across steps ----
        h_sb = h0
        hT = h0T
        xT = x0T
        S_prev = None
        vals_prev = None
        gmax_prev = None

        for t in range(T_DEC):
            # ================= GRU =================
            g_rz = psA.tile([B, 1024], F32, name="g_rz", tag="bigps")
            gn2 = psA.tile([B, 1024], F32, name="gn2", tag="bigps")  # [gi_n | gh_n]
            # gh first (depends only on prev h -> overlaps prev step tail),
            # gi last (depends on this step's gathered x).
            for j in range(2):
                o_sl = slice(j * 512, (j + 1) * 512)
                for k in range(4):
                    nc.tensor.matmul(
                        g_rz[:, o_sl], r32(hT[:, k * B:(k + 1) * B]),
                        r32(whhT[:, k * 1536 + j * 512: k * 1536 + (j + 1) * 512]),
                        start=(k == 0), stop=False)
                for k in range(2):
                    nc.tensor.matmul(
                        g_rz[:, o_sl], r32(xT[:, k * B:(k + 1) * B]),
                        r32(wihT[:, k * 1536 + j * 512: k * 1536 + (j + 1) * 512]),
                        start=False, stop=(k == 1))
            for k in range(4):   # gh_n -> gn2[:, 512:1024]
                nc.tensor.matmul(gn2[:, 512:1024], r32(hT[:, k * B:(k + 1) * B]),
                                 r32(whhT[:, k * 1536 + 1024: k * 1536 + 1536]),
                                 start=(k == 0), stop=(k == 3))
            for k in range(2):   # gi_n -> gn2[:, 0:512]
                nc.tensor.matmul(gn2[:, 0:512], r32(xT[:, k * B:(k + 1) * B]),
                                 r32(wihT[:, k * 1536 + 1024: k * 1536 + 1536]),
                                 start=(k == 0), stop=(k == 1))

            grz = w2([B, 1024], "grz")
            nc.vector.tensor_add(grz[:], g_rz[:], bias_rz[:])
            r_g = w2([B, H], "r_g")
            z_g = w2([B, H], "z_g")
            nc.scalar.activation(r_g[:], grz[:, 0:512], AF.Tanh, scale=0.5)
            nc.scalar.activation(z_g[:], grz[:, 512:1024], AF.Tanh, scale=0.5)
            nc.vector.tensor_scalar(r_g[:], r_g[:], 0.5, 0.5, ALU.mult, ALU.add)
            nc.vector.tensor_scalar(z_g[:], z_g[:], 0.5, 0.5, ALU.mult, ALU.add)
            t1 = w2([B, H], "t1")
            nc.vector.tensor_add(t1[:], gn2[:, 512:1024], bias_hn[:])
            nc.vector.tensor_mul(t1[:], r_g[:], t1[:])
            nc.vector.tensor_add(t1[:], t1[:], gn2[:, 0:512])
            nc.vector.tensor_add(t1[:], t1[:], bias_in[:])
            n_g = w2([B, H], "n_g")
            nc.scalar.activation(n_g[:], t1[:], AF.Tanh)
            d1 = wrk.tile([B, H], F32, name="d1", tag="t1", bufs=1)
            nc.vector.tensor_sub(d1[:], h_sb[:], n_g[:])
            nc.vector.tensor_mul(d1[:], z_g[:], d1[:])
            h_new = w2([B, H], "h_new", bufs=2)
            nc.vector.tensor_add(h_new[:], n_g[:], d1[:])
            h_sb = h_new

            hT_n = w2([128, 4 * B], "hT_n", bufs=2)
            for k in range(4):
                pe_tr(h_new[:, k * 128:(k + 1) * 128], B, 128,
                      hT_n[:, k * B:(k + 1) * B])
            hT = hT_n

            # ================= attention (batch shard) =================
            q_ps = psA.tile([B, EH], F32, name="q_ps", tag="bigps")
            for k in range(4):
                nc.tensor.matmul(q_ps[:], r32(hT[:, k * B:(k + 1) * B]),
                                 r32(waattn[:, k * EH:(k + 1) * EH]),
                                 start=(k == 0), stop=(k == 3))
            q_sb = wrk.tile([B, EH], F32, name="q_sb", tag="mid2k", bufs=1)
            nc.vector.tensor_copy(q_sb[:], q_ps[:])

            qsT = w2([128, 4 * BS], "qsT")
            hsT = w2([128, 4 * BS], "hsT")
            for src, dst in ((q_sb, qsT), (h_new, hsT)):
                for m in range(4):
                    ps = small_ps(128, BS)
                    nc.tensor.matmul(ps, r32(src[:, m * 128:(m + 1) * 128]),
                                     r32(sel[:]), start=True, stop=True)
                    nc.vector.tensor_copy(dst[:, m * BS:(m + 1) * BS], ps)

            # scores as a block matmul out[b, (b',t)]; off-diagonal blocks are
            # masked to -1e30 so the softmax zeroes them exactly.
            sc_ps = psA.tile([BS, BS * T_ENC], F32, name="sc_ps", tag="bigps")
            for j in range(2):
                o_sl = slice(j * 512, (j + 1) * 512)
                for k in range(4):
                    nc.tensor.matmul(
                        sc_ps[:, o_sl], r32(qsT[:, k * BS:(k + 1) * BS]),
                        r32(encselT[:, k * 1024 + j * 512: k * 1024 + (j + 1) * 512]),
                        start=(k == 0), stop=(k == 3))
            scores = w2([BS, BS * T_ENC], "scores")
            nc.vector.tensor_add(scores[:], sc_ps[:], scmask[:])
            smax = w2([BS, 8], "smax")
            nc.vector.max(smax[:], scores[:])
            negm = w2([BS, 1], "negm")
            nc.vector.tensor_scalar(negm[:], smax[:, 0:1], -1.0, None, ALU.mult)
            attn = scores
            ssum = w2([BS, 1], "ssum")
            nc.scalar.activation(attn[:], scores[:], AF.Exp, bias=negm[:],
                                 accum_out=ssum[:])
            srec = w2([BS, 1], "srec")
            nc.vector.reciprocal(srec[:], ssum[:])
            nc.vector.tensor_scalar(attn[:], attn[:], srec[:], None, ALU.mult)
            # transpose each 128-chunk: chunk b gives [T_ENC, BS] whose col b is
            # attn_b and all other cols are exact zeros -> block-diag attn^T.
            attnT = w2([T_ENC, BS * BS], "attnT")
            for b in range(BS):
                pe_tr(attn[:, b * T_ENC:(b + 1) * T_ENC], BS, T_ENC,
                      attnT[:, b * BS:(b + 1) * BS])

            ctx_ps = small_ps(BS, EH)
            for b in range(BS):
                nc.tensor.matmul(ctx_ps, r32(attnT[:, b * BS:(b + 1) * BS]),
                                 r32(encselN[:, b * EH:(b + 1) * EH]),
                                 start=(b == 0), stop=(b == BS - 1))
            ctx_sb = wrk.tile([BS, EH], F32, name="ctx_sb", tag="small2k", bufs=1)
            nc.vector.tensor_copy(ctx_sb[:], ctx_ps)
            ctxT = w2([128, 4 * BS], "ctxT")
            for k in range(4):
                pe_tr(ctx_sb[:, k * 128:(k + 1) * 128], BS, 128,
                      ctxT[:, k * BS:(k + 1) * BS])

            o_ps = small_ps(BS, H)
            for k in range(8):
                lhs = hsT if k < 4 else ctxT
                nc.tensor.matmul(o_ps, r32(lhs[:, (k % 4) * BS:(k % 4 + 1) * BS]),
                                 r32(waT[:, k * H:(k + 1) * H]),
                                 start=(k == 0), stop=(k == 7))
            o_sb = wrk.tile([BS, H], F32, name="o_sb", tag="small2k", bufs=1)
            nc.scalar.activation(o_sb[:], o_ps, AF.Tanh)

            # ================= AllGather #1: o shards =================
            ag1_in = drm.tile([BS, H], F32, name="ag1_in", tag="ag1_in")
            ag1_out = drm.tile([B, H], F32, name="ag1_out", tag="ag1_out",
                               addr_space="Shared")
            nc.sync.dma_start(ag1_in[:], o_sb[:])
            nc.gpsimd.collective_compute("AllGather", ALU.bypass, replica_groups=RG,
                                         ins=[ag1_in.opt()], outs=[ag1_out.opt()])
            ofull = wrk.tile([B, H], F32, name="ofull", tag="mid2k", bufs=1)
            nc.sync.dma_start(ofull[:], ag1_out[:])
            oT = w2([128, 4 * B], "oT")
            for k in range(4):
                pe_tr(ofull[:, k * 128:(k + 1) * 128], B, 128,
                      oT[:, k * B:(k + 1) * B])

            # ================= z = o @ fc_shard^T (chunked) =================
            runmax = w2([B, 1], "runmax", bufs=2)
            runidx = w2([B, 1], "runidx", bufs=2)
            negmjs = w2([B, NZC], "negmjs")
            Sp = w2([B, NZC], "Sp")
            for j in range(NZC):
                z_ps = small_ps(B, ZC)
                for k in range(4):
                    nc.tensor.matmul(z_ps, r32(oT[:, k * B:(k + 1) * B]),
                                     r32(fcT[:, k * VS + j * ZC: k * VS + (j + 1) * ZC]),
                                     start=(k == 0), stop=(k == 3))
                zc = w2([B, ZC], "zc", bufs=2)
                nc.vector.tensor_copy(zc[:], z_ps)
                nc.sync.dma_start(zraw_d.ap()[t, :, j * ZC:(j + 1) * ZC], zc[:])
                m8 = w2([B, 8], "m8", bufs=2)
                nc.vector.max(m8[:], zc[:])
                i8 = w2([B, 8], "i8", U32, bufs=2)
                nc.vector.max_index(i8[:], m8[:], zc[:])
                ijf = w2([B, 1], "ijf", bufs=2)
                nc.vector.tensor_copy(ijf[:], i8[:, 0:1])
                nc.vector.tensor_scalar(ijf[:], ijf[:], voff[:], float(j * ZC),
                                        ALU.add, ALU.add)
                nc.vector.tensor_scalar(negmjs[:, j:j + 1], m8[:, 0:1], -1.0, None,
                                        ALU.mult)
                nc.scalar.activation(zc[:], zc[:], AF.Exp, bias=negmjs[:, j:j + 1],
                                     accum_out=Sp[:, j:j + 1])
                if j == 0:
                    nc.vector.tensor_copy(runmax[:], m8[:, 0:1])
                    nc.vector.tensor_copy(runidx[:], ijf[:])
                else:
                    btr = w2([B, 1], "btr", mybir.dt.uint8, bufs=2)
                    nc.vector.tensor_tensor(btr[:], m8[:, 0:1], runmax[:], ALU.is_gt)
                    nc.vector.copy_predicated(runmax[:], btr[:], m8[:, 0:1])
                    nc.vector.copy_predicated(runidx[:], btr[:], ijf[:])
            neglm = w2([B, 1], "neglm")
            nc.vector.tensor_scalar(neglm[:], runmax[:], -1.0, None, ALU.mult)
            wj = w2([B, NZC], "wj")
            nc.scalar.activation(wj[:], negmjs[:], AF.Exp, bias=neglm[:], scale=-1.0)
            nc.vector.tensor_mul(wj[:], wj[:], Sp[:])
            S_t = w2([B, 1], "S_t", bufs=2)
            nc.vector.tensor_reduce(S_t[:], wj[:], AX.X, ALU.add)

            # ============ AllGather #2: argmax partials + S_{t-1} ============
            pk = w2([B, 4], "pk")
            nc.vector.tensor_copy(pk[:, 0:1], runmax[:])
            nc.vector.tensor_copy(pk[:, 1:2], runidx[:])
            if S_prev is not None:
                nc.vector.tensor_copy(pk[:, 2:3], S_prev[:])
            else:
                nc.vector.memset(pk[:, 2:3], 1.0)
            nc.vector.memset(pk[:, 3:4], 0.0)
            ag2_in = drm.tile([B, 4], F32, name="ag2_in", tag="ag2_in")
            ag2_out = drm.tile([NCORE * B, 4], F32, name="ag2_out", tag="ag2_out",
                               addr_space="Shared")
            nc.sync.dma_start(ag2_in[:], pk[:])
            nc.gpsimd.collective_compute("AllGather", ALU.bypass, replica_groups=RG,
                                         ins=[ag2_in.opt()], outs=[ag2_out.opt()])
            cand_cb = w2([B, 4 * NCORE], "cand_cb", bufs=2)
            nc.sync.dma_start(
                cand_cb[:], ag2_out.rearrange("(c b) k -> b c k", c=NCORE))
            cand = w2([B, 3 * NCORE], "cand", bufs=2)
            nc.vector.tensor_copy(
                cand[:].rearrange("b (k c) -> b k c", c=NCORE),
                cand_cb[:].rearrange("b (c k) -> b k c", k=4)[:, 0:3, :])
            vals = cand[:, 0:8]
            idxs = cand[:, 8:16]
            sumexps = cand[:, 16:24]

            gm8 = w2([B, 8], "gm8")
            nc.vector.max(gm8[:], vals)
            gmax = w2([B, 1], "gmax", bufs=2)
            nc.vector.tensor_copy(gmax[:], gm8[:, 0:1])
            eqm = w2([B, NCORE], "eqm")
            nc.vector.tensor_scalar(eqm[:], vals, gmax[:], None, ALU.is_equal)
            mi = w2([B, NCORE], "mi")
            nc.vector.tensor_mul(mi[:], eqm[:], idxs)
            nc.vector.tensor_scalar(eqm[:], eqm[:], -1e9, 1e9, ALU.mult, ALU.add)
            nc.vector.tensor_add(mi[:], mi[:], eqm[:])
            gidx = w2([B, 1], "gidx")
            nc.vector.tensor_reduce(gidx[:], mi[:], AX.X, ALU.min)
            tok = w2([B, 1], "tok", U32, bufs=2)
            nc.vector.tensor_copy(tok[:], gidx[:])

            # ---- deferred logZ + normalization of step t-1 ----
            if t > 0:
                normalize_block(t - 1, vals_prev, gmax_prev[:], sumexps, "")

            vals_keep = w2([B, NCORE], "vals_keep", bufs=2)
            nc.vector.tensor_copy(vals_keep[:], vals)
            vals_prev = vals_keep[:]
            gmax_prev = gmax
            S_prev = S_t

            # ---- token gather + x^T for next step ----
            if t + 1 < T_DEC:
                x_sb = w2([B, E], "x_sb", bufs=1)
                nc.gpsimd.indirect_dma_start(
                    out=x_sb[:], out_offset=None, in_=emb_d.ap(),
                    in_offset=IndirectOffsetOnAxis(ap=tok[:, :1], axis=0))
                xT_n = w2([128, 2 * B], "xT_n", bufs=2)
                for k in range(2):
                    pe_tr(x_sb[:, k * 128:(k + 1) * 128], B, 128,
                          xT_n[:, k * B:(k + 1) * B])
                xT = xT_n

        # -------- post-loop: combine S of last step, normalize z_{T-1} --------
        pk2 = w2([B, 4], "pk2")
        nc.vector.tensor_copy(pk2[:, 2:3], S_prev[:])
        nc.vector.memset(pk2[:, 0:2], 0.0)
        nc.vector.memset(pk2[:, 3:4], 0.0)
        ag3_in = drm.tile([B, 4], F32, name="ag3_in", tag="ag2_in")
        ag3_out = drm.tile([NCORE * B, 4], F32, name="ag3_out", tag="ag2_out",
                           addr_space="Shared")
        nc.sync.dma_start(ag3_in[:], pk2[:])
        nc.gpsimd.collective_compute("AllGather", mybir.AluOpType.bypass,
                                     replica_groups=RG,
                                     ins=[ag3_in.opt()], outs=[ag3_out.opt()])
        cand2_cb = w2([B, 4 * NCORE], "cand_cb", bufs=2)
        nc.sync.dma_start(
            cand2_cb[:], ag3_out.rearrange("(c b) k -> b c k", c=NCORE))
        cand2 = w2([B, 3 * NCORE], "cand2", bufs=2)
        nc.vector.tensor_copy(
            cand2[:].rearrange("b (k c) -> b k c", c=NCORE),
            cand2_cb[:].rearrange("b (c k) -> b k c", k=4)[:, 0:3, :])
        normalize_block(T_DEC - 1, vals_prev, gmax_prev[:], cand2[:, 16:24], "f")

        nc.sync.dma_start(hfin_d.ap()[:], h_sb[:])


def _prepare_inputs(encoder_hidden, encoder_outputs, emb, w_ih, w_hh, b_ih, b_hh,
                    wa_attn, wa, fc):
    f = lambda a: np.ascontiguousarray(np.asarray(a, dtype=np.float32))
    emb = f(emb); w_ih = f(w_ih); w_hh = f(w_hh); b_ih = f(b_ih); b_hh = f(b_hh)
    wa_attn = f(wa_attn); wa = f(wa); fc = f(fc)
    enc = f(encoder_outputs)
    h0 = f(encoder_hidden)[0]

    whhT = _kmajor(w_hh.T.copy())
    wihT = _kmajor(w_ih.T.copy())
    waattn = _kmajor(wa_attn)
    waT = _kmajor(wa.T.copy())
    bias_rz = np.ascontiguousarray(
        np.broadcast_to((b_ih + b_hh)[:1024], (B, 1024))).astype(np.float32)
    bias_hn = np.ascontiguousarray(
        np.broadcast_to(b_hh[1024:], (B, H))).astype(np.float32)
    bias_in = np.ascontiguousarray(
        np.broadcast_to(b_ih[1024:], (B, H))).astype(np.float32)
    ident = np.eye(128, dtype=np.float32)
    h0T = _kmajor(h0.T.copy())
    x0 = np.ascontiguousarray(np.broadcast_to(emb[SOS], (B, E))).astype(np.float32)
    x0T = _kmajor(x0.T.copy())

    in_maps = []
    for c in range(NCORE):
        bsl = slice(c * BS, (c + 1) * BS)
        enc_sh = enc[bsl]                                   # [BS, T_ENC, EH]
        eT = enc_sh.transpose(2, 0, 1).reshape(4, 128, BS * T_ENC)
        encselT = np.ascontiguousarray(
            eT.transpose(1, 0, 2).reshape(128, 4 * BS * T_ENC))
        encselN = np.ascontiguousarray(
            enc_sh.transpose(1, 0, 2).reshape(T_ENC, BS * EH))
        fcT = _kmajor(fc[c * VS:(c + 1) * VS].T.copy())
        scmask = np.full((BS, BS * T_ENC), -1e30, dtype=np.float32)
        for j in range(BS):
            scmask[j, j * T_ENC:(j + 1) * T_ENC] = 0.0
        sel_c = np.zeros((B, BS), dtype=np.float32)
        for j in range(BS):
            sel_c[c * BS + j, j] = 1.0
        voff = np.full((B, 1), float(c * VS), dtype=np.float32)
        in_maps.append({
            "emb": emb, "encselT": encselT, "encselN": encselN, "fcT": fcT,
            "whhT": whhT, "wihT": wihT, "waattn": waattn, "waT": waT,
            "bias_rz": bias_rz, "bias_hn": bias_hn, "bias_in": bias_in,
            "ident": ident, "sel": sel_c, "voff": voff, "scmask": scmask,
            "h0": h0, "h0T": h0T, "x0T": x0T,
        })
    return in_maps


_BUILT = {}


def _get_nc(T_DEC):
    if T_DEC not in _BUILT:
        nc = bacc.Bacc("TRN2", target_bir_lowering=False, debug=False,
                       num_devices=NCORE)
        _emit(nc, T_DEC)
        nc.compile()
        _BUILT[T_DEC] = nc
    return _BUILT[T_DEC]


def kernel(encoder_hidden, encoder_outputs, target, emb, w_ih, w_hh, b_ih, b_hh,
           wa_attn, wa, fc, _t_dec=None, _trace=False):
    global _LAST_RESULTS
    T_DEC = _t_dec if _t_dec is not None else int(os.environ.get("NN_TDEC", "129"))
    in_maps = _prepare_inputs(encoder_hidden, encoder_outputs, emb, w_ih, w_hh,
                              b_ih, b_hh, wa_attn, wa, fc)
    nc = _get_nc(T_DEC)
    res = run_bass_kernel_spmd(nc, in_maps, core_ids=list(range(NCORE)),
                               trace=_trace)
    _LAST_RESULTS = res
    parts = [res.results[c]["logp"] for c in range(NCORE)]
    decoder_outputs = np.concatenate(parts, axis=2).astype(np.float32)
    decoder_hidden = res.results[0]["hfin"][None]
    return decoder_outputs, decoder_hidden


# revision 14
# speedup vs baseline: 1.0318x; 1.0318x over previous
"""Trainium2 Bass kernel for nn_Decoder (GRU decoder + Luong attention + greedy decode).

Strategy (8 NeuronCores, SPMD, same NEFF, per-core data):
  - GRU + tanh-output replicated for the full batch B=64 (matmul streams are
    B-independent); attention batch-sharded 8 ways (dual enc layouts per core);
    vocab projection sharded 8 ways (fc rows 4000/core, float32r matmuls).
  - Per step two tiny AllGathers: (1) o_t shards [8,512] -> full o_t, (2) local
    argmax partials (+ deferred softmax stats) -> greedy token feedback.
  - log_softmax normalization is deferred one step and runs off the critical
    path; ln() is computed on DVE via exponent/mantissa split + Chebyshev
    polynomial so the ACT engine only ever uses the exp/tanh table set.
"""

import os
from contextlib import ExitStack

import numpy as np

import concourse.bass as bass
import concourse.bacc as bacc
import concourse.mybir as mybir
from concourse.tile import TileContext
from concourse.bass import IndirectOffsetOnAxis
from concourse.bass_utils import run_bass_kernel_spmd

F32 = mybir.dt.float32
F32R = mybir.dt.float32r
U32 = mybir.dt.uint32

V = 32000
E = 256
H = 512
EH = 512
B = 64
T_ENC = 128
SOS = 2
NCORE = 8
BS = B // NCORE        # 8 batches per core (attention shard)
VS = V // NCORE        # 4000 vocab rows per core
ZC = 500               # z free-dim chunk (VS = 8 * ZC)
NZC = VS // ZC
LN2 = 0.6931471805599453

_LAST_RESULTS = None   # BassKernelResults of the most recent run (for test.py)


def _log2_cheb_coeffs(deg=12):
    # Chebyshev fit of log2 on [1, 2] -> power-series coeffs (ascending in f).
    x = np.cos(np.pi * (np.arange(200) + 0.5) / 200)
    xs = 1.5 + 0.5 * x
    c = np.polynomial.chebyshev.chebfit(x, np.log2(xs), deg)
    p = np.polynomial.chebyshev.cheb2poly(c)
    q = np.polynomial.polynomial.Polynomial(p)(
        np.polynomial.polynomial.Polynomial([-3.0, 2.0]))
    return q.coef


def _kmajor(w):
    """[K, N] (K multiple of 128) -> [128, (K//128)*N]; chunk k at cols [k*N:(k+1)*N]."""
    K, N = w.shape
    assert K % 128 == 0
    return np.ascontiguousarray(
        w.reshape(K // 128, 128, N).transpose(1, 0, 2).reshape(128, (K // 128) * N)
    ).astype(np.float32)


def _emit(nc, T_DEC):
    AF = mybir.ActivationFunctionType
    ALU = mybir.AluOpType
    AX = mybir.AxisListType
    RG = [list(range(NCORE))]

    ei = lambda name, shape, d=F32: nc.dram_tensor(name, shape, d, kind="ExternalInput")
    emb_d = ei("emb", [V, E])
    encselT_d = ei("encselT", [128, 4 * BS * T_ENC])
    encselN_d = ei("encselN", [128, BS * EH])
    fcT_d = ei("fcT", [128, 4 * VS])
    whhT_d = ei("whhT", [128, 4 * 1536])
    wihT_d = ei("wihT", [128, 2 * 1536])
    waattn_d = ei("waattn", [128, 4 * EH])
    waT_d = ei("waT", [128, 8 * H])
    bias_rz_d = ei("bias_rz", [B, 1024])
    bias_hn_d = ei("bias_hn", [B, H])
    bias_in_d = ei("bias_in", [B, H])
    ident_d = ei("ident", [128, 128])
    sel_d = ei("sel", [B, BS])
    scmask_d = ei("scmask", [BS, BS * T_ENC])
    voff_d = ei("voff", [B, 1])
    h0_d = ei("h0", [B, H])
    h0T_d = ei("h0T", [128, 4 * B])
    x0T_d = ei("x0T", [128, 2 * B])

    logp_d = nc.dram_tensor("logp", [B, T_DEC, VS], F32, kind="ExternalOutput")
    hfin_d = nc.dram_tensor("hfin", [B, H], F32, kind="ExternalOutput")
    zraw_d = nc.dram_tensor("zraw", [T_DEC, B, VS], F32, kind="Internal")

    r32 = lambda ap: ap  # fp32r is TF32 (10-bit mantissa) - too imprecise here
    cheb = _log2_cheb_coeffs(12)[::-1]  # descending for Horner

    with TileContext(nc) as tc, ExitStack() as ctx:
        cst = ctx.enter_context(tc.tile_pool(name="cst", bufs=1))
        wrk = ctx.enter_context(tc.tile_pool(name="wrk", bufs=1))
        psA = ctx.enter_context(tc.tile_pool(name="psA", bufs=2, space="PSUM"))
        psB = ctx.enter_context(tc.tile_pool(name="psB", bufs=3, space="PSUM"))
        drm = ctx.enter_context(tc.tile_pool(name="drm", bufs=2, space="DRAM"))

        def load_const(dram, shape):
            t = cst.tile(shape, F32, name=dram.name + "_sb")
            nc.sync.dma_start(t[:], dram.ap())
            return t

        encselT = load_const(encselT_d, [128, 4 * BS * T_ENC])
        encselN = load_const(encselN_d, [128, BS * EH])
        fcT = load_const(fcT_d, [128, 4 * VS])
        whhT = load_const(whhT_d, [128, 4 * 1536])
        wihT = load_const(wihT_d, [128, 2 * 1536])
        waattn = load_const(waattn_d, [128, 4 * EH])
        waT = load_const(waT_d, [128, 8 * H])
        bias_rz = load_const(bias_rz_d, [B, 1024])
        bias_hn = load_const(bias_hn_d, [B, H])
        bias_in = load_const(bias_in_d, [B, H])
        ident = load_const(ident_d, [128, 128])
        sel = load_const(sel_d, [B, BS])
        scmask = load_const(scmask_d, [BS, BS * T_ENC])
        voff = load_const(voff_d, [B, 1])
        h0 = wrk.tile([B, H], F32, name="h_new", tag="h_new", bufs=2)
        nc.sync.dma_start(h0[:], h0_d.ap())
        h0T = wrk.tile([128, 4 * B], F32, name="hT_n", tag="hT_n", bufs=2)
        nc.sync.dma_start(h0T[:], h0T_d.ap())
        x0T = wrk.tile([128, 2 * B], F32, name="xT_n", tag="xT_n", bufs=2)
        nc.sync.dma_start(x0T[:], x0T_d.ap())

        # small-psum slot: shared by transposes / sel / ctx / o / z chunks
        def small_ps(p, f):
            t = psB.tile([128, 512], F32, name="sps", tag="sps")
            return t[:p, :f]

        def w2(shape, name, dtype=F32, bufs=1):
            return wrk.tile(shape, dtype, name=name, tag=name, bufs=bufs)

        def pe_tr(src_ap, p, f, dst_ap):
            """dst[f-cols...] = src[p,f].T via PE; dst is an SBUF slice [f, p]."""
            ps = small_ps(f, p)
            nc.tensor.transpose(ps, src_ap, ident[:p, :p])
            nc.vector.tensor_copy(dst_ap, ps)

        def ln_into(out_ap, s_ap, uniq):
            """out = ln(s) elementwise on [B,1] via bits + Chebyshev log2."""
            ubits = w2([B, 1], "ubits" + uniq, U32)
            nc.vector.tensor_copy(ubits[:].bitcast(F32), s_ap)
            eu = w2([B, 1], "eu" + uniq, U32)
            nc.vector.tensor_scalar(eu[:], ubits[:], 23, None,
                                    ALU.logical_shift_right)
            ef = w2([B, 1], "ef" + uniq)
            nc.vector.tensor_copy(ef[:], eu[:])
            mu = w2([B, 1], "mu" + uniq, U32)
            nc.vector.tensor_scalar(mu[:], ubits[:], 0x007FFFFF, 0x3F800000,
                                    ALU.bitwise_and, ALU.bitwise_or)
            mf = mu[:].bitcast(F32)
            nc.vector.tensor_scalar(out_ap, mf, float(cheb[0]), float(cheb[1]),
                                    ALU.mult, ALU.add)
            for c in cheb[2:]:
                nc.vector.tensor_scalar(out_ap, out_ap, mf, float(c),
                                        ALU.mult, ALU.add)
            nc.vector.tensor_add(out_ap, out_ap, ef[:])
            nc.vector.tensor_scalar(out_ap, out_ap, -127.0, LN2, ALU.add, ALU.mult)

        def normalize_block(t_src, vals_prev_ap, gmax_prev_ap, sumexps_ap, uniq):
            """logp[:, t_src, :] = zraw[t_src] - (gmax + ln(sum_c S_c e^{lm_c-gmax}))."""
            adj = w2([B, NCORE], "adj" + uniq)
            ngp = w2([B, 1], "ngp" + uniq)
            nc.vector.tensor_scalar(ngp[:], gmax_prev_ap, -1.0, None, ALU.mult)
            nc.scalar.activation(adj[:], vals_prev_ap, AF.Exp, bias=ngp[:])
            nc.vector.tensor_mul(adj[:], adj[:], sumexps_ap)
            stot = w2([B, 1], "stot" + uniq)
            nc.vector.tensor_reduce(stot[:], adj[:], AX.X, ALU.add)
            logz = w2([B, 1], "logz" + uniq)
            ln_into(logz[:], stot[:], uniq)
            nc.vector.tensor_add(logz[:], logz[:], gmax_prev_ap)
            for jn in range(8):
                zn = w2([B, 500], "zn", bufs=1)
                nc.sync.dma_start(
                    zn[:], zraw_d.ap()[t_src, :, jn * 500:(jn + 1) * 500])
                nc.vector.tensor_scalar(zn[:], zn[:], logz[:], None, ALU.subtract)
                nc.sync.dma_start(
                    logp_d.ap()[:, t_src, jn * 500:(jn + 1) * 500], zn[:])

        # ---- state carried across steps ----
        h_sb = h0
        hT = h0T
        xT = x0T
        S_prev = None
        vals_prev = None
        gmax_prev = None

        for t in range(T_DEC):
            # ================= GRU =================
            # col-packed: K-chunk group G uses PE column half G and writes
            # psum rows [64G:64G+64]; halves sum on DVE afterwards.
            g_rz = psA.tile([128, 1024], F32, name="g_rz", tag="bigps")
            gn2 = psA.tile([128, 1024], F32, name="gn2", tag="bigps")  # [gi_n | gh_n]
            for j in range(2):
                o_sl = slice(j * 512, (j + 1) * 512)
                for G in range(2):
                    rows = slice(64 * G, 64 * G + 64)
                    for k in (2 * G, 2 * G + 1):   # gh first (prev-h dep only)
                        nc.tensor.matmul(
                            g_rz[rows, o_sl], hT[:, k * B:(k + 1) * B],
                            whhT[:, k * 1536 + j * 512: k * 1536 + (j + 1) * 512],
                            start=(k == 2 * G), stop=False,
                            tile_position=(0, 64 * G))
                    nc.tensor.matmul(
                        g_rz[rows, o_sl], xT[:, G * B:(G + 1) * B],
                        wihT[:, G * 1536 + j * 512: G * 1536 + (j + 1) * 512],
                        start=False, stop=True, tile_position=(0, 64 * G))
            for G in range(2):   # gh_n -> gn2[rows, 512:1024]
                rows = slice(64 * G, 64 * G + 64)
                for k in (2 * G, 2 * G + 1):
                    nc.tensor.matmul(gn2[rows, 512:1024], hT[:, k * B:(k + 1) * B],
                                     whhT[:, k * 1536 + 1024: k * 1536 + 1536],
                                     start=(k == 2 * G), stop=(k == 2 * G + 1),
                                     tile_position=(0, 64 * G))
            for G in range(2):   # gi_n -> gn2[rows, 0:512]
                rows = slice(64 * G, 64 * G + 64)
                nc.tensor.matmul(gn2[rows, 0:512], xT[:, G * B:(G + 1) * B],
                                 wihT[:, G * 1536 + 1024: G * 1536 + 1536],
                                 start=True, stop=True, tile_position=(0, 64 * G))

            grz = w2([B, 1024], "grz")
            nc.vector.tensor_add(grz[:], g_rz[0:64, :], bias_rz[:])
            nc.vector.tensor_add(grz[:], grz[:], g_rz[64:128, :])
            r_g = w2([B, H], "r_g")
            z_g = w2([B, H], "z_g")
            nc.scalar.activation(r_g[:], grz[:, 0:512], AF.Tanh, scale=0.5)
            nc.scalar.activation(z_g[:], grz[:, 512:1024], AF.Tanh, scale=0.5)
            nc.vector.tensor_scalar(r_g[:], r_g[:], 0.5, 0.5, ALU.mult, ALU.add)
            nc.vector.tensor_scalar(z_g[:], z_g[:], 0.5, 0.5, ALU.mult, ALU.add)
            t1 = w2([B, H], "t1")
            nc.vector.tensor_add(t1[:], gn2[0:64, 512:1024], bias_hn[:])
            nc.vector.tensor_add(t1[:], t1[:], gn2[64:128, 512:1024])
            nc.vector.tensor_mul(t1[:], r_g[:], t1[:])
            nc.vector.tensor_add(t1[:], t1[:], gn2[0:64, 0:512])
            nc.vector.tensor_add(t1[:], t1[:], bias_in[:])
            nc.vector.tensor_add(t1[:], t1[:], gn2[64:128, 0:512])
            n_g = w2([B, H], "n_g")
            nc.scalar.activation(n_g[:], t1[:], AF.Tanh)
            d1 = wrk.tile([B, H], F32, name="d1", tag="t1", bufs=1)
            nc.vector.tensor_sub(d1[:], h_sb[:], n_g[:])
            nc.vector.tensor_mul(d1[:], z_g[:], d1[:])
            h_new = w2([B, H], "h_new", bufs=2)
            nc.vector.tensor_add(h_new[:], n_g[:], d1[:])
            h_sb = h_new

            hT_n = w2([128, 4 * B], "hT_n", bufs=2)
            for k in range(4):
                pe_tr(h_new[:, k * 128:(k + 1) * 128], B, 128,
                      hT_n[:, k * B:(k + 1) * B])
            hT = hT_n

            # ================= attention (batch shard) =================
            q_ps = psA.tile([128, EH], F32, name="q_ps", tag="bigps")
            for G in range(2):
                rows = slice(64 * G, 64 * G + 64)
                for k in (2 * G, 2 * G + 1):
                    nc.tensor.matmul(q_ps[rows, :], hT[:, k * B:(k + 1) * B],
                                     waattn[:, k * EH:(k + 1) * EH],
                                     start=(k == 2 * G), stop=(k == 2 * G + 1),
                                     tile_position=(0, 64 * G))
            q_sb = wrk.tile([B, EH], F32, name="q_sb", tag="mid2k", bufs=1)
            nc.vector.tensor_copy(q_sb[:], q_ps[0:64, :])
            nc.vector.tensor_add(q_sb[:], q_sb[:], q_ps[64:128, :])

            qsT = w2([128, 4 * BS], "qsT")
            hsT = w2([128, 4 * BS], "hsT")
            for src, dst in ((q_sb, qsT), (h_new, hsT)):
                for m in range(4):
                    ps = small_ps(128, BS)
                    nc.tensor.matmul(ps, r32(src[:, m * 128:(m + 1) * 128]),
                                     r32(sel[:]), start=True, stop=True)
                    nc.vector.tensor_copy(dst[:, m * BS:(m + 1) * BS], ps)

            # scores as a block matmul out[b, (b',t)]; off-diagonal blocks are
            # masked to -1e30 so the softmax zeroes them exactly.
            sc_ps = psA.tile([BS, BS * T_ENC], F32, name="sc_ps", tag="bigps")
            for j in range(2):
                o_sl = slice(j * 512, (j + 1) * 512)
                for k in range(4):
                    nc.tensor.matmul(
                        sc_ps[:, o_sl], r32(qsT[:, k * BS:(k + 1) * BS]),
                        r32(encselT[:, k * 1024 + j * 512: k * 1024 + (j + 1) * 512]),
                        start=(k == 0), stop=(k == 3))
            scores = w2([BS, BS * T_ENC], "scores")
            nc.vector.tensor_add(scores[:], sc_ps[:], scmask[:])
            smax = w2([BS, 8], "smax")
            nc.vector.max(smax[:], scores[:])
            negm = w2([BS, 1], "negm")
            nc.vector.tensor_scalar(negm[:], smax[:, 0:1], -1.0, None, ALU.mult)
            attn = scores
            ssum = w2([BS, 1], "ssum")
            nc.scalar.activation(attn[:], scores[:], AF.Exp, bias=negm[:],
                                 accum_out=ssum[:])
            srec = w2([BS, 1], "srec")
            nc.vector.reciprocal(srec[:], ssum[:])
            nc.vector.tensor_scalar(attn[:], attn[:], srec[:], None, ALU.mult)
            # transpose each 128-chunk: chunk b gives [T_ENC, BS] whose col b is
            # attn_b and all other cols are exact zeros -> block-diag attn^T.
            attnT = w2([T_ENC, BS * BS], "attnT")
            for b in range(BS):
                pe_tr(attn[:, b * T_ENC:(b + 1) * T_ENC], BS, T_ENC,
                      attnT[:, b * BS:(b + 1) * BS])

            ctx_ps = small_ps(BS, EH)
            for b in range(BS):
                nc.tensor.matmul(ctx_ps, r32(attnT[:, b * BS:(b + 1) * BS]),
                                 r32(encselN[:, b * EH:(b + 1) * EH]),
                                 start=(b == 0), stop=(b == BS - 1))
            ctx_sb = wrk.tile([BS, EH], F32, name="ctx_sb", tag="small2k", bufs=1)
            nc.vector.tensor_copy(ctx_sb[:], ctx_ps)
            ctxT = w2([128, 4 * BS], "ctxT")
            for k in range(4):
                pe_tr(ctx_sb[:, k * 128:(k + 1) * 128], BS, 128,
                      ctxT[:, k * BS:(k + 1) * BS])

            o_ps_full = psB.tile([128, 512], F32, name="sps", tag="sps")
            for G in range(2):
                rows = slice(64 * G, 64 * G + BS)
                for kk in range(4):
                    k = G * 4 + kk
                    lhs = hsT if k < 4 else ctxT
                    nc.tensor.matmul(o_ps_full[rows, :],
                                     lhs[:, (k % 4) * BS:(k % 4 + 1) * BS],
                                     waT[:, k * H:(k + 1) * H],
                                     start=(kk == 0), stop=(kk == 3),
                                     tile_position=(0, 64 * G))
            o_pre = wrk.tile([BS, H], F32, name="o_pre", tag="o_pre", bufs=1)
            nc.vector.tensor_copy(o_pre[:], o_ps_full[0:BS, :])
            nc.vector.tensor_add(o_pre[:], o_pre[:], o_ps_full[64:64 + BS, :])
            o_sb = wrk.tile([BS, H], F32, name="o_sb", tag="small2k", bufs=1)
            nc.scalar.activation(o_sb[:], o_pre[:], AF.Tanh)

            # ================= AllGather #1: o shards =================
            ag1_in = drm.tile([BS, H], F32, name="ag1_in", tag="ag1_in")
            ag1_out = drm.tile([B, H], F32, name="ag1_out", tag="ag1_out",
                               addr_space="Shared")
            nc.sync.dma_start(ag1_in[:], o_sb[:])
            nc.gpsimd.collective_compute("AllGather", ALU.bypass, replica_groups=RG,
                                         ins=[ag1_in.opt()], outs=[ag1_out.opt()])
            ofull = wrk.tile([B, H], F32, name="ofull", tag="mid2k", bufs=1)
            nc.sync.dma_start(ofull[:], ag1_out[:])
            oT = w2([128, 4 * B], "oT")
            for k in range(4):
                pe_tr(ofull[:, k * 128:(k + 1) * 128], B, 128,
                      oT[:, k * B:(k + 1) * B])

            # ================= z = o @ fc_shard^T (chunked) =================
            runmax = w2([B, 1], "runmax", bufs=2)
            runidx = w2([B, 1], "runidx", bufs=2)
            negmjs = w2([B, NZC], "negmjs")
            Sp = w2([B, NZC], "Sp")
            for j in range(NZC):
                z_ps = psB.tile([128, 512], F32, name="sps", tag="sps")
                for G in range(2):
                    rows = slice(64 * G, 64 * G + 64)
                    for k in (2 * G, 2 * G + 1):
                        nc.tensor.matmul(
                            z_ps[rows, :ZC], oT[:, k * B:(k + 1) * B],
                            fcT[:, k * VS + j * ZC: k * VS + (j + 1) * ZC],
                            start=(k == 2 * G), stop=(k == 2 * G + 1),
                            tile_position=(0, 64 * G))
                zc = w2([B, ZC], "zc", bufs=2)
                nc.vector.tensor_copy(zc[:], z_ps[0:64, :ZC])
                nc.vector.tensor_add(zc[:], zc[:], z_ps[64:128, :ZC])
                nc.sync.dma_start(zraw_d.ap()[t, :, j * ZC:(j + 1) * ZC], zc[:])
                m8 = w2([B, 8], "m8", bufs=2)
                nc.vector.max(m8[:], zc[:])
                i8 = w2([B, 8], "i8", U32, bufs=2)
                nc.vector.max_index(i8[:], m8[:], zc[:])
                ijf = w2([B, 1], "ijf", bufs=2)
                nc.vector.tensor_copy(ijf[:], i8[:, 0:1])
                nc.vector.tensor_scalar(ijf[:], ijf[:], voff[:], float(j * ZC),
                                        ALU.add, ALU.add)
                nc.vector.tensor_scalar(negmjs[:, j:j + 1], m8[:, 0:1], -1.0, None,
                                        ALU.mult)
                nc.scalar.activation(zc[:], zc[:], AF.Exp, bias=negmjs[:, j:j + 1],
                                     accum_out=Sp[:, j:j + 1])
                if j == 0:
                    nc.vector.tensor_copy(runmax[:], m8[:, 0:1])
                    nc.vector.tensor_copy(runidx[:], ijf[:])
                else:
                    btr = w2([B, 1], "btr", mybir.dt.uint8, bufs=2)
                    nc.vector.tensor_tensor(btr[:], m8[:, 0:1], runmax[:], ALU.is_gt)
                    nc.vector.copy_predicated(runmax[:], btr[:], m8[:, 0:1])
                    nc.vector.copy_predicated(runidx[:], btr[:], ijf[:])
            neglm = w2([B, 1], "neglm")
            nc.vector.tensor_scalar(neglm[:], runmax[:], -1.0, None, ALU.mult)
            wj = w2([B, NZC], "wj")
            nc.scalar.activation(wj[:], negmjs[:], AF.Exp, bias=neglm[:], scale=-1.0)
            nc.vector.tensor_mul(wj[:], wj[:], Sp[:])
            S_t = w2([B, 1], "S_t", bufs=2)
            nc.vector.tensor_reduce(S_t[:], wj[:], AX.X, ALU.add)

            # ============ AllGather #2: argmax partials + S_{t-1} ============
            pk = w2([B, 4], "pk")
            nc.vector.tensor_copy(pk[:, 0:1], runmax[:])
            nc.vector.tensor_copy(pk[:, 1:2], runidx[:])
            if S_prev is not None:
                nc.vector.tensor_copy(pk[:, 2:3], S_prev[:])
            else:
                nc.vector.memset(pk[:, 2:3], 1.0)
            nc.vector.memset(pk[:, 3:4], 0.0)
            ag2_in = drm.tile([B, 4], F32, name="ag2_in", tag="ag2_in")
            ag2_out = drm.tile([NCORE * B, 4], F32, name="ag2_out", tag="ag2_out",
                               addr_space="Shared")
            nc.sync.dma_start(ag2_in[:], pk[:])
            nc.gpsimd.collective_compute("AllGather", ALU.bypass, replica_groups=RG,
                                         ins=[ag2_in.opt()], outs=[ag2_out.opt()])
            cand_cb = w2([B, 4 * NCORE], "cand_cb", bufs=2)
            nc.sync.dma_start(
                cand_cb[:], ag2_out.rearrange("(c b) k -> b c k", c=NCORE))
            cand = w2([B, 3 * NCORE], "cand", bufs=2)
            nc.vector.tensor_copy(
                cand[:].rearrange("b (k c) -> b k c", c=NCORE),
                cand_cb[:].rearrange("b (c k) -> b k c", k=4)[:, 0:3, :])
            vals = cand[:, 0:8]
            idxs = cand[:, 8:16]
            sumexps = cand[:, 16:24]

            gm8 = w2([B, 8], "gm8")
            nc.vector.max(gm8[:], vals)
            gmax = w2([B, 1], "gmax", bufs=2)
            nc.vector.tensor_copy(gmax[:], gm8[:, 0:1])
            eqm = w2([B, NCORE], "eqm")
            nc.vector.tensor_scalar(eqm[:], vals, gmax[:], None, ALU.is_equal)
            mi = w2([B, NCORE], "mi")
            nc.vector.tensor_mul(mi[:], eqm[:], idxs)
            nc.vector.tensor_scalar(eqm[:], eqm[:], -1e9, 1e9, ALU.mult, ALU.add)
            nc.vector.tensor_add(mi[:], mi[:], eqm[:])
            gidx = w2([B, 1], "gidx")
            nc.vector.tensor_reduce(gidx[:], mi[:], AX.X, ALU.min)
            tok = w2([B, 1], "tok", U32, bufs=2)
            nc.vector.tensor_copy(tok[:], gidx[:])

            # ---- deferred logZ + normalization of step t-1 ----
            if t > 0:
                normalize_block(t - 1, vals_prev, gmax_prev[:], sumexps, "")

            vals_keep = w2([B, NCORE], "vals_keep", bufs=2)
            nc.vector.tensor_copy(vals_keep[:], vals)
            vals_prev = vals_keep[:]
            gmax_prev = gmax
            S_prev = S_t

            # ---- token gather + x^T for next step ----
            if t + 1 < T_DEC:
                x_sb = w2([B, E], "x_sb", bufs=1)
                nc.gpsimd.indirect_dma_start(
                    out=x_sb[:], out_offset=None, in_=emb_d.ap(),
                    in_offset=IndirectOffsetOnAxis(ap=tok[:, :1], axis=0))
                xT_n = w2([128, 2 * B], "xT_n", bufs=2)
                for k in range(2):
                    pe_tr(x_sb[:, k * 128:(k + 1) * 128], B, 128,
                          xT_n[:, k * B:(k + 1) * B])
                xT = xT_n

        # -------- post-loop: combine S of last step, normalize z_{T-1} --------
        pk2 = w2([B, 4], "pk2")
        nc.vector.tensor_copy(pk2[:, 2:3], S_prev[:])
        nc.vector.memset(pk2[:, 0:2], 0.0)
        nc.vector.memset(pk2[:, 3:4], 0.0)
        ag3_in = drm.tile([B, 4], F32, name="ag3_in", tag="ag2_in")
        ag3_out = drm.tile([NCORE * B, 4], F32, name="ag3_out", tag="ag2_out",
                           addr_space="Shared")
        nc.sync.dma_start(ag3_in[:], pk2[:])
        nc.gpsimd.collective_compute("AllGather", mybir.AluOpType.bypass,
                                     replica_groups=RG,
                                     ins=[ag3_in.opt()], outs=[ag3_out.opt()])
        cand2_cb = w2([B, 4 * NCORE], "cand_cb", bufs=2)
        nc.sync.dma_start(
            cand2_cb[:], ag3_out.rearrange("(c b) k -> b c k", c=NCORE))
        cand2 = w2([B, 3 * NCORE], "cand2", bufs=2)
        nc.vector.tensor_copy(
            cand2[:].rearrange("b (k c) -> b k c", c=NCORE),
            cand2_cb[:].rearrange("b (c k) -> b k c", k=4)[:, 0:3, :])
        normalize_block(T_DEC - 1, vals_prev, gmax_prev[:], cand2[:, 16:24], "f")

        nc.sync.dma_start(hfin_d.ap()[:], h_sb[:])


def _prepare_inputs(encoder_hidden, encoder_outputs, emb, w_ih, w_hh, b_ih, b_hh,
                    wa_attn, wa, fc):
    f = lambda a: np.ascontiguousarray(np.asarray(a, dtype=np.float32))
    emb = f(emb); w_ih = f(w_ih); w_hh = f(w_hh); b_ih = f(b_ih); b_hh = f(b_hh)
    wa_attn = f(wa_attn); wa = f(wa); fc = f(fc)
    enc = f(encoder_outputs)
    h0 = f(encoder_hidden)[0]

    whhT = _kmajor(w_hh.T.copy())
    wihT = _kmajor(w_ih.T.copy())
    waattn = _kmajor(wa_attn)
    waT = _kmajor(wa.T.copy())
    bias_rz = np.ascontiguousarray(
        np.broadcast_to((b_ih + b_hh)[:1024], (B, 1024))).astype(np.float32)
    bias_hn = np.ascontiguousarray(
        np.broadcast_to(b_hh[1024:], (B, H))).astype(np.float32)
    bias_in = np.ascontiguousarray(
        np.broadcast_to(b_ih[1024:], (B, H))).astype(np.float32)
    ident = np.eye(128, dtype=np.float32)
    h0T = _kmajor(h0.T.copy())
    x0 = np.ascontiguousarray(np.broadcast_to(emb[SOS], (B, E))).astype(np.float32)
    x0T = _kmajor(x0.T.copy())

    in_maps = []
    for c in range(NCORE):
        bsl = slice(c * BS, (c + 1) * BS)
        enc_sh = enc[bsl]                                   # [BS, T_ENC, EH]
        eT = enc_sh.transpose(2, 0, 1).reshape(4, 128, BS * T_ENC)
        encselT = np.ascontiguousarray(
            eT.transpose(1, 0, 2).reshape(128, 4 * BS * T_ENC))
        encselN = np.ascontiguousarray(
            enc_sh.transpose(1, 0, 2).reshape(T_ENC, BS * EH))
        fcT = _kmajor(fc[c * VS:(c + 1) * VS].T.copy())
        scmask = np.full((BS, BS * T_ENC), -1e30, dtype=np.float32)
        for j in range(BS):
            scmask[j, j * T_ENC:(j + 1) * T_ENC] = 0.0
        sel_c = np.zeros((B, BS), dtype=np.float32)
        for j in range(BS):
            sel_c[c * BS + j, j] = 1.0
        voff = np.full((B, 1), float(c * VS), dtype=np.float32)
        in_maps.append({
            "emb": emb, "encselT": encselT, "encselN": encselN, "fcT": fcT,
            "whhT": whhT, "wihT": wihT, "waattn": waattn, "waT": waT,
            "bias_rz": bias_rz, "bias_hn": bias_hn, "bias_in": bias_in,
            "ident": ident, "sel": sel_c, "voff": voff, "scmask": scmask,
            "h0": h0, "h0T": h0T, "x0T": x0T,
        })
    return in_maps


_BUILT = {}


def _get_nc(T_DEC):
    if T_DEC not in _BUILT:
        nc = bacc.Bacc("TRN2", target_bir_lowering=False, debug=False,
                       num_devices=NCORE)
        _emit(nc, T_DEC)
        nc.compile()
        _BUILT[T_DEC] = nc
    return _BUILT[T_DEC]


def kernel(encoder_hidden, encoder_outputs, target, emb, w_ih, w_hh, b_ih, b_hh,
           wa_attn, wa, fc, _t_dec=None, _trace=False):
    global _LAST_RESULTS
    T_DEC = _t_dec if _t_dec is not None else int(os.environ.get("NN_TDEC", "129"))
    in_maps = _prepare_inputs(encoder_hidden, encoder_outputs, emb, w_ih, w_hh,
                              b_ih, b_hh, wa_attn, wa, fc)
    nc = _get_nc(T_DEC)
    res = run_bass_kernel_spmd(nc, in_maps, core_ids=list(range(NCORE)),
                               trace=_trace)
    _LAST_RESULTS = res
    parts = [res.results[c]["logp"] for c in range(NCORE)]
    decoder_outputs = np.concatenate(parts, axis=2).astype(np.float32)
    decoder_hidden = res.results[0]["hfin"][None]
    return decoder_outputs, decoder_hidden
